# revision 16
# baseline (speedup 1.0000x reference)
# Trainium2 Bass kernel for nn_DirectRanker (ragged_sequence).
#
# Math shortcut: result = tanh((sorted_enc[:,1:,:] - sorted_enc[:,:1,:]) @ W.T)
# commutes with the linear map, so per-row scores s = encodes @ W.T are
# computed FIRST (the memory-bound part: 1 GiB streamed once), and the
# per-group sort/diff/tanh runs on the tiny [N] score vector:
#   result[g, k-1] = tanh(s_sorted[g, k] - s_sorted[g, 0]),  k = 1..63
#
# Sharding: groups split across 8 cores (2048 groups/core), no cross-core
# communication. Inside a core, E is DMA'd so that partition p holds rows of
# group (T*128 + p): the fused multiply-reduce (DVE tensor_tensor_reduce)
# then yields scores directly in [group(partition), elem(free)] layout, so
# no transpose is ever needed.
#
# Exact stable argsort over y within each 64-row group: integer keys
#   key = (y * 2^23 + 2^23) * 64 | elem_index     (y is a multiple of 2^-23)
# are compared through an f32 bitcast view (monotone for positive int32;
# keys lie in [2^29, 2^30) so the views are normal floats). rank[i] =
# sum_j (key_j < key_i) via tensor_scalar accumulate passes; the scatter
# s_sorted[k] = sum_i (rank_i == k) * s_i via scalar_tensor_tensor passes.
import os
from contextlib import ExitStack

import numpy as np

import concourse.bacc as bacc
import concourse.mybir as mybir
import concourse.tile as tile
from concourse.bass_utils import run_bass_kernel_spmd

N_CORES = 8
N = 1048576
D = 256
G = 64
NG = N // G                # 16384 groups
ROWS = N // N_CORES        # 131072 rows per core
GPC = NG // N_CORES        # 2048 groups per core
T_TILES = GPC // 128       # 16 tiles of 128 groups per core
UB = 8                     # u-rows per E DMA (1 MiB transfers)
PE_MOD = 8                 # u % PE_MOD == PE_MOD-1 rows go via TensorE
F32 = mybir.dt.float32
I32 = mybir.dt.int32
I16 = mybir.dt.int16
Alu = mybir.AluOpType

_built = {}


def _build_nc():
    nc = bacc.Bacc("TRN2", target_bir_lowering=False, debug=False,
                   num_devices=N_CORES)
    e_in = nc.dram_tensor("encodes", [ROWS, D], F32, kind="ExternalInput")
    y_in = nc.dram_tensor("y_coord", [ROWS], F32, kind="ExternalInput")
    w_in = nc.dram_tensor("w", [1, D], F32, kind="ExternalInput")
    out = nc.dram_tensor("result", [GPC * (G - 1)], F32, kind="ExternalOutput")

    # [T, p, u, d]: partition p of tile T holds the G rows of group T*128+p
    e_r = e_in.ap().rearrange("(t p u) d -> t p u d", p=128, u=G)
    y_r = y_in.ap().rearrange("(t p u) -> t p u", p=128, u=G)
    out_r = out.ap().rearrange("(t p k) -> t p k", p=128, k=G - 1)

    with tile.TileContext(nc) as tc, ExitStack() as ctx:
        const_pool = ctx.enter_context(tc.tile_pool(name="const", bufs=1))
        epool = ctx.enter_context(tc.tile_pool(name="e", bufs=12))
        spool = ctx.enter_context(tc.tile_pool(name="s", bufs=6))
        scr_pool = ctx.enter_context(tc.tile_pool(name="scr", bufs=4))
        etsb_pool = ctx.enter_context(tc.tile_pool(name="etsb", bufs=3))
        pt_pool = ctx.enter_context(
            tc.tile_pool(name="pt", bufs=3, space="PSUM"))
        ps_pool = ctx.enter_context(
            tc.tile_pool(name="ps", bufs=2, space="PSUM"))

        wb = const_pool.tile([128, D], F32)
        nc.sync.dma_start(wb[:], w_in.ap()[0, :].partition_broadcast(128))
        iota_i = const_pool.tile([128, G], I32)
        nc.gpsimd.iota(iota_i[:], pattern=[[1, G]], base=0, channel_multiplier=0)
        # descending iota (63..0) as int16: data for the rank-producing scatter
        iota_d16 = const_pool.tile([128, G], I16)
        nc.gpsimd.iota(iota_d16[:], pattern=[[-1, G]], base=G - 1,
                       channel_multiplier=0)
        # identity matrix for TensorE transposes + W with d on partitions
        iota128 = const_pool.tile([128, 128], I32)
        nc.gpsimd.iota(iota128[:], pattern=[[1, 128]], base=0,
                       channel_multiplier=0)
        iota128f = const_pool.tile([128, 128], F32)
        nc.vector.tensor_copy(iota128f[:], iota128[:])
        pidx = const_pool.tile([128, 1], I32)
        nc.gpsimd.iota(pidx[:], pattern=[[0, 1]], base=0, channel_multiplier=1)
        pidxf = const_pool.tile([128, 1], F32)
        nc.vector.tensor_copy(pidxf[:], pidx[:])
        ident = const_pool.tile([128, 128], F32)
        nc.vector.tensor_scalar(out=ident[:], in0=iota128f[:],
                                scalar1=pidxf[:, 0:1], scalar2=None,
                                op0=Alu.is_equal)
        wsb = const_pool.tile([128, D // 128], F32)
        nc.sync.dma_start(wsb[:],
                          w_in.ap()[0, :].rearrange("(c p) -> p c", p=128))

        for T in range(T_TILES):
            # --- keys from y ---
            y_t = spool.tile([128, G], F32, tag="y")
            nc.sync.dma_start(y_t[:], y_r[T])
            ki = spool.tile([128, G], I32, tag="ki")
            nc.scalar.activation(ki[:], y_t[:],
                                 mybir.ActivationFunctionType.Copy,
                                 bias=float(1 << 23), scale=float(1 << 23))
            k64 = spool.tile([128, G], I32, tag="k64")
            nc.scalar.activation(k64[:], ki[:],
                                 mybir.ActivationFunctionType.Copy,
                                 bias=0.0, scale=64.0)
            keys = spool.tile([128, G], I32, tag="keys")
            nc.vector.tensor_tensor(out=keys[:], in0=k64[:], in1=iota_i[:],
                                    op=Alu.bitwise_or)
            keys_f = keys[:].bitcast(F32)

            # --- full descending sort of the int keys on DVE via 8 rounds of
            # max8 + match_replace (compares run on the f32 bitcast views,
            # which order identically to the positive int32 keys) ---
            sorted_i = spool.tile([128, G], I32, tag="sorted")
            wka = scr_pool.tile([128, G], I32, tag="wka")
            wkb = scr_pool.tile([128, G], I32, tag="wkb")
            src = keys
            dst = wka
            for r in range(8):
                nc.vector.max(sorted_i[:, r * 8:(r + 1) * 8].bitcast(F32),
                              src[:].bitcast(F32))
                if r < 7:
                    nc.vector.match_replace(
                        dst[:].bitcast(F32),
                        sorted_i[:, r * 8:(r + 1) * 8].bitcast(F32),
                        src[:].bitcast(F32), 0.0)
                    src, dst = dst, (wkb if dst is wka else wka)

            # perm (descending argsort) = low 6 bits of the sorted keys
            perm32 = scr_pool.tile([128, G], I32, tag="perm32")
            nc.vector.tensor_scalar(out=perm32[:], in0=sorted_i[:], scalar1=63,
                                    scalar2=None, op0=Alu.bitwise_and)
            perm16 = spool.tile([128, G], I16, tag="perm16")
            nc.vector.tensor_copy(perm16[:], perm32[:])
            # rank_asc[i] = position of element i in ascending order:
            # scatter descending iota by perm
            rank16 = spool.tile([128, G], I16, tag="rank16")
            nc.gpsimd.local_scatter(rank16[:], iota_d16[:], perm16[:],
                                    channels=128, num_elems=G, num_idxs=G)

            # --- scores: s[p, u] = dot(E[group row u], W) ---
            # most rows via DVE fused multiply-accumulate; every PE_MOD-th row
            # via TensorE (transpose -> Act copy -> fp32 matvec into PSUM)
            s_t = spool.tile([128, G], F32, tag="s")
            n_pe = G // PE_MOD
            psum_s = ps_pool.tile([128, n_pe], F32, tag="psum_s")
            for u0 in range(0, G, UB):
                e_t = epool.tile([128, UB, D], F32, tag="e")
                nc.sync.dma_start(e_t[:], e_r[T, :, u0:u0 + UB, :])
                for ul in range(UB):
                    u = u0 + ul
                    if u % PE_MOD == PE_MOD - 1:
                        pt = pt_pool.tile([128, D], F32, tag="pt")
                        for c in range(D // 128):
                            nc.tensor.transpose(
                                pt[:, c * 128:(c + 1) * 128],
                                e_t[:, ul, c * 128:(c + 1) * 128], ident[:])
                        etsb = etsb_pool.tile([128, D], F32, tag="etsb")
                        nc.scalar.copy(etsb[:], pt[:])
                        j = u // PE_MOD
                        for c in range(D // 128):
                            nc.tensor.matmul(
                                psum_s[:, j:j + 1],
                                etsb[:, c * 128:(c + 1) * 128],
                                wsb[:, c:c + 1],
                                start=(c == 0), stop=(c == D // 128 - 1))
                    else:
                        prod = scr_pool.tile([128, D], F32, tag="prod")
                        nc.vector.scalar_tensor_tensor(
                            out=prod[:], in0=e_t[:, ul, :], scalar=1.0,
                            in1=wb[:], op0=Alu.mult, op1=Alu.mult,
                            accum_out=s_t[:, u:u + 1])
            # collect the PE-computed scores into s_t (strided columns)
            s_t_pe = s_t[:].rearrange("p (a b) -> p a b", b=PE_MOD)
            nc.scalar.copy(s_t_pe[:, :, PE_MOD - 1:PE_MOD],
                           psum_s[:].unsqueeze(-1))

            # --- permute scores by rank on gpsimd: f32 as two int16 halves ---
            s16 = s_t[:].bitcast(I16).rearrange("p (i two) -> p i two", two=2)
            lo16 = scr_pool.tile([128, G], I16, tag="lo16")
            hi16 = scr_pool.tile([128, G], I16, tag="hi16")
            nc.scalar.copy(lo16[:].unsqueeze(-1), s16[:, :, 0:1])
            nc.scalar.copy(hi16[:].unsqueeze(-1), s16[:, :, 1:2])
            slo = scr_pool.tile([128, G], I16, tag="slo")
            shi = scr_pool.tile([128, G], I16, tag="shi")
            nc.gpsimd.local_scatter(slo[:], lo16[:], rank16[:],
                                    channels=128, num_elems=G, num_idxs=G)
            nc.gpsimd.local_scatter(shi[:], hi16[:], rank16[:],
                                    channels=128, num_elems=G, num_idxs=G)
            ssort = spool.tile([128, G], F32, tag="ssort")
            o16 = ssort[:].bitcast(I16).rearrange("p (i two) -> p i two", two=2)
            nc.scalar.copy(o16[:, :, 0:1], slo[:].unsqueeze(-1))
            nc.scalar.copy(o16[:, :, 1:2], shi[:].unsqueeze(-1))

            # --- result tile: tanh(ssort[:, 1:] - ssort[:, 0]) ---
            negs0 = spool.tile([128, 1], F32, tag="negs0")
            nc.scalar.mul(negs0[:], ssort[:, 0:1], -1.0)
            th = spool.tile([128, G - 1], F32, tag="th")
            nc.scalar.activation(th[:], ssort[:, 1:G],
                                 mybir.ActivationFunctionType.Tanh,
                                 bias=negs0[:], scale=1.0)
            nc.sync.dma_start(out_r[T], th[:])

    nc.compile()
    return nc


last_results = None


def kernel(encodes, y_coord, W, x_coord=None):
    global last_results
    if "nc" not in _built:
        _built["nc"] = _build_nc()
    nc = _built["nc"]

    encodes = np.ascontiguousarray(np.asarray(encodes, dtype=np.float32))
    y_coord = np.ascontiguousarray(np.asarray(y_coord, dtype=np.float32))
    W = np.ascontiguousarray(np.asarray(W, dtype=np.float32))

    in_maps = []
    for c in range(N_CORES):
        in_maps.append({
            "encodes": encodes[c * ROWS:(c + 1) * ROWS],
            "y_coord": y_coord[c * ROWS:(c + 1) * ROWS],
            "w": W,
        })
    res = run_bass_kernel_spmd(
        nc, in_maps, core_ids=list(range(N_CORES)),
        trace=bool(os.environ.get("BASS_TRACE")),
    )
    last_results = res
    result = np.concatenate([r["result"] for r in res.results])
    polarity = np.ones(NG * (G - 1), dtype=np.float32)
    return result, polarity
